# revision 1
# baseline (speedup 1.0000x reference)
"""GroupWhitening1d Trainium2 kernel.

x: [16384, 4096] f32, G=32 groups of d=128.
  out = (x - mean) @ blockdiag(W_g),  W_g = U_g S_g^-1/2 U_g^T from eigh of
  per-group covariance.

Strategy (data-parallel over rows, 8 cores x 2048 rows):
  K1 (device): SWDGE cast-loads each 128-row tile of the f32 shard into a
      PERSISTENT fp16 SBUF cache (16MB/core, survives across NEFF runs);
      fp16 Gram matmuls per group with f32 PSUM accumulation (all 8 banks).
  Host: reduce grams over cores, mean via numpy, cov, eigh (f64), W;
      pack W (fp16) and bias b = -(mu_g W_g) broadcast (f32).
  K2 (device): zero input traffic — reads the resident SBUF cache;
      PE-transposes each [128,128] group block (PSUM), ScalarE evacuates,
      fp16 matmul with W_g, DVE evacuates adding the bias (centers the
      output), stores fp16; host casts back to f32.
"""

import sys
import numpy as np

if "/opt/trn_rl_repo" not in sys.path:
    sys.path.insert(0, "/opt/trn_rl_repo")

N, D, G, d = 16384, 4096, 32, 128
NCORES = 8
NS = N // NCORES  # rows per core

_built = {}


def _build_k1(ns=NS):
    from concourse import bacc, mybir, tile

    f16, f32 = mybir.dt.float16, mybir.dt.float32
    nt = ns // 128
    nc = bacc.Bacc(None, target_bir_lowering=False)
    x = nc.dram_tensor("x", [ns, D], f32, kind="ExternalInput")
    # layout [bank, d, gsub, e]; host: transpose(0,2,1,3).reshape(G,d,d)
    gram = nc.dram_tensor("gram", [8, 128, 4, 128], f32, kind="ExternalOutput")
    cache = nc.alloc_sbuf_tensor("xtc", [128, nt * D], f16)
    with tile.TileContext(nc) as tc:
        with (
            tc.tile_pool(name="ev", bufs=2) as ev,
            tc.tile_pool(name="ps", bufs=8, space="PSUM") as ps,
        ):
            gp = [
                ps.tile([128, 512], f32, tag="gram", name=f"gram{b}")
                for b in range(8)
            ]
            for t in range(nt):
                csl = cache.ap()[:, t * D:(t + 1) * D]
                # SWDGE cast-load f32 -> fp16 straight into the resident cache
                nc.gpsimd.dma_start(csl, x[t * 128:(t + 1) * 128, :])
                for g in range(G):
                    b, s = divmod(g, 4)
                    xg = cache.ap()[:, t * D + g * 128: t * D + (g + 1) * 128]
                    # one accumulation group per PSUM bank: start zeroes the
                    # whole 2KB zero region, so only the first slice starts
                    nc.tensor.matmul(
                        gp[b][:, s * 128:(s + 1) * 128],
                        xg,
                        xg,
                        start=(t == 0 and s == 0),
                        stop=(t == nt - 1 and s == 3),
                    )
            for b in range(8):
                e = ev.tile([128, 512], f32, tag="ev")
                if b % 2 == 0:
                    nc.vector.tensor_copy(e[:], gp[b][:])
                else:
                    nc.scalar.activation(
                        e[:], gp[b][:], mybir.ActivationFunctionType.Copy
                    )
                nc.sync.dma_start(gram[b], e[:])
    nc.compile()
    return nc


def _build_k2(ns=NS):
    from concourse import bacc, mybir, tile

    f16, f32 = mybir.dt.float16, mybir.dt.float32
    nt = ns // 128
    nc = bacc.Bacc(None, target_bir_lowering=False)
    wp = nc.dram_tensor("wp", [128, D], f16, kind="ExternalInput")
    bb = nc.dram_tensor("bb", [128, D], f32, kind="ExternalInput")
    idn = nc.dram_tensor("idn", [128, 128], f16, kind="ExternalInput")
    out = nc.dram_tensor("out", [ns, D], f16, kind="ExternalOutput")
    # must match _build_k1's allocation exactly (same name/shape/order)
    cache = nc.alloc_sbuf_tensor("xtc", [128, nt * D], f16)
    with tile.TileContext(nc) as tc:
        with (
            tc.tile_pool(name="cp", bufs=1) as cp,
            tc.tile_pool(name="xqp", bufs=4) as xqp,
            tc.tile_pool(name="otp", bufs=3) as otp,
            tc.tile_pool(name="ptp", bufs=3, space="PSUM") as ptp,
            tc.tile_pool(name="pop", bufs=3, space="PSUM") as pop,
        ):
            wps = cp.tile([128, D], f16, tag="wp")
            nc.sync.dma_start(wps[:], wp[:])
            bbs = cp.tile([128, D], f32, tag="bb")
            nc.sync.dma_start(bbs[:], bb[:])
            ids = cp.tile([128, 128], f16, tag="idn")
            nc.sync.dma_start(ids[:], idn[:])
            for t in range(nt):
                ot = otp.tile([128, D], f16, tag="ot")
                for q in range(G // 4):
                    tq = ptp.tile([128, 512], f16, tag="tq")
                    for k in range(4):
                        g = q * 4 + k
                        nc.tensor.matmul(
                            tq[:, k * 128:(k + 1) * 128],
                            cache.ap()[:, t * D + g * 128: t * D + (g + 1) * 128],
                            ids[:],
                            is_transpose=True,
                            start=(k == 0),
                            stop=(k == 3),
                        )
                    xq = xqp.tile([128, 512], f16, tag="xq")
                    nc.scalar.activation(
                        xq[:], tq[:], mybir.ActivationFunctionType.Copy
                    )
                    oq = pop.tile([128, 512], f32, tag="oq")
                    for k in range(4):
                        g = q * 4 + k
                        nc.tensor.matmul(
                            oq[:, k * 128:(k + 1) * 128],
                            xq[:, k * 128:(k + 1) * 128],
                            wps[:, g * 128:(g + 1) * 128],
                            start=(k == 0),
                            stop=(k == 3),
                        )
                    # bias add performs the centering: out = xW - (mu W)
                    nc.vector.tensor_add(
                        out=ot[:, q * 512:(q + 1) * 512],
                        in0=oq[:],
                        in1=bbs[:, q * 512:(q + 1) * 512],
                    )
                nc.sync.dma_start(out[t * 128:(t + 1) * 128, :], ot[:])
    nc.compile()
    return nc


def _sbuf_addr(nc, name):
    for a in nc.m.functions[0].allocations:
        if hasattr(a, "memorylocations") and a.memorylocations:
            ml = a.memorylocations[0]
            if ml.name == name:
                return getattr(ml, "addr", None)
    return None


def _host_solve(gram, mu64):
    """gram: [G,d,d] f64 raw sum of x_g^T x_g; mu64: [D] f64."""
    mug = mu64.reshape(G, d)
    cov = (gram - N * np.einsum("gd,ge->gde", mug, mug)) / (N - 1)
    cov = (cov + cov.transpose(0, 2, 1)) / 2
    S, U = np.linalg.eigh(cov)
    S = np.maximum(S, 1e-12)
    W = np.einsum("gde,ge,gfe->gdf", U, 1.0 / np.sqrt(S), U)
    return W  # [G, d, d]


def kernel(x):
    from concourse.bass_utils import run_bass_kernel_spmd

    x = np.ascontiguousarray(x, dtype=np.float32)
    core_ids = list(range(NCORES))
    shards = [x[c * NS:(c + 1) * NS] for c in range(NCORES)]

    if "k1" not in _built:
        _built["k1"] = _build_k1()
    if "k2" not in _built:
        _built["k2"] = _build_k2()
        a1 = _sbuf_addr(_built["k1"], "xtc")
        a2 = _sbuf_addr(_built["k2"], "xtc")
        assert a1 == a2 and a1 is not None, (a1, a2)

    r1 = run_bass_kernel_spmd(_built["k1"], [{"x": s} for s in shards], core_ids)
    gram = np.zeros((G, d, d), np.float64)
    for r in r1.results:
        gram += r["gram"].astype(np.float64).transpose(0, 2, 1, 3).reshape(G, d, d)

    mu64 = x.mean(axis=0, dtype=np.float64)
    W = _host_solve(gram, mu64)

    wpk = np.ascontiguousarray(
        W.transpose(1, 0, 2).reshape(d, G * d).astype(np.float16)
    )
    bvec = -np.einsum("gd,gdf->gf", mu64.reshape(G, d), W).reshape(D)
    bbb = np.ascontiguousarray(
        np.broadcast_to(bvec.astype(np.float32), (128, D))
    )
    idn = np.eye(128, dtype=np.float16)

    in2 = [{"wp": wpk, "bb": bbb, "idn": idn} for _ in shards]
    global _last_in2
    _last_in2 = in2
    r2 = run_bass_kernel_spmd(_built["k2"], in2, core_ids)
    return np.concatenate(
        [r["out"].astype(np.float32) for r in r2.results], axis=0
    )

